# revision 73
# baseline (speedup 1.0000x reference)
"""NetGINE (4-layer GIN message passing) on 8 Trainium2 NeuronCores.

Sharding: nodes/edges sharded by destination across 8 cores (6400 padded node
slots per core).  The replicated node table in HBM uses a quarter-major layout
[quarter][core][local] so each quarter of a layer's output can be AllGathered
as soon as its chunks finish, overlapping the collective with the edge phase.

Per layer, per 4-tile chunk (fused edge + node pipeline):
  - h[src] gathered per edge from the replicated f32 table via gpsimd
    dma_gather (4 SWDGE queues, table split in two for int16 idx)
  - bond encoder e = relu(ea @ be1) @ be2 on TensorE (2-group stacked chain)
  - msg = relu(h_src + e); one-hot scatter (flipped: one-hot stationary,
    msg streamed 64-wide) accumulates natural-layout agg per dst tile
  - node update for the chunk's tiles: z natural -> bf16 transposes ->
    bf16 MLP -> BN -> transpose back; bounce DMA (scalar queue)
  - graph pooling per tile against a host-built [nodes, 128] local-graph
    matrix with 1/cnt baked in, accumulated in a dedicated PSUM bank
Quarter AllGathers are emitted a few chunks after their tiles complete;
epilogue: place pooled locals into [64, 512], AllReduce, head MLP on
every core.
"""

import os
import numpy as np
import ml_dtypes

BF16 = np.dtype(ml_dtypes.bfloat16)

N, E, G, DIM, XF, EF = 50000, 800000, 512, 64, 28, 3
NCORES = 8
NLOC = 6400              # padded node slots per core
NPAD = NCORES * NLOC     # 51200
TILES = NLOC // 128      # 50
TPC = 4                  # tiles per chunk
NLAYERS = 4

# quarter-major table layout: quarters are chunk-aligned tile ranges
HALF_A = NPAD // 2                          # 25600: gather half A boundary


def _pad_id(core, slot):
    return core * NLOC + slot


# ---------------------------------------------------------------- host prep --

def _plan_nodes(batch):
    """Assign nodes to per-core padded slots; no 128-slot tile may span a
    128-graph window boundary."""
    slot2node = np.full((NCORES, NLOC), -1, np.int64)
    node2pad = np.full(N, -1, np.int64)
    per_core = N // NCORES  # 6250
    for c in range(NCORES):
        nodes = np.arange(c * per_core, (c + 1) * per_core)
        wins = batch[nodes] // 128
        change = np.nonzero(np.diff(wins))[0] + 1
        bounds = [0] + list(change) + [len(nodes)]
        s = 0
        for i in range(len(bounds) - 1):
            lo, hi = bounds[i], bounds[i + 1]
            if i > 0 and s % 128 != 0:
                s += 128 - (s % 128)
            cnt = hi - lo
            assert s + cnt <= NLOC, "node padding overflow"
            slot2node[c, s:s + cnt] = nodes[lo:hi]
            node2pad[nodes[lo:hi]] = _pad_id(c, s + np.arange(cnt))
            s += cnt
    return slot2node, node2pad


def _prep(inputs):
    x = np.asarray(inputs["x"], np.float32)
    edge_attr = np.asarray(inputs["edge_attr"], np.float32)
    edge_index = np.asarray(inputs["edge_index"], np.int64)
    batch = np.asarray(inputs["batch"], np.int64)

    slot2node, node2pad = _plan_nodes(batch)

    src_p = node2pad[edge_index[0]]
    dst_p = node2pad[edge_index[1]]
    core = dst_p // NLOC
    dslot = dst_p % NLOC
    drel = dslot % 128
    tile_of = dslot // 128
    grp_b = (src_p >= HALF_A).astype(np.int64)   # 0 = A, 1 = B

    key = (core * TILES + tile_of) * 2 + grp_b
    counts = np.bincount(key, minlength=NCORES * TILES * 2).reshape(NCORES, TILES, 2)
    BA = max(int(np.ceil(counts[:, :, 0].max() / 128)), 1)
    BB = max(int(np.ceil(counts[:, :, 1].max() / 128)), 1)
    if (BA + BB) % 2 == 1:
        BB += 1
    NBT = BA + BB
    NBLK = TILES * NBT
    SLOTS = NBLK * 128
    assert BA <= 15 and BB <= 15, (BA, BB)

    chunk_tiles = [[0, 1]] + [list(range(t, min(t + TPC, TILES)))
                              for t in range(2, TILES, TPC)]

    slot_base = {}
    s0 = 0
    for tl in chunk_tiles:
        for ti in tl:
            slot_base[(ti, 0)] = s0
            s0 += BA * 128
        for ti in tl:
            slot_base[(ti, 1)] = s0
            s0 += BB * 128
    assert s0 == SLOTS

    # slot of each edge: base of its (tile, group) + rank within that list
    order = np.argsort(key, kind="stable")
    ends = np.cumsum(counts.reshape(-1))
    starts = ends - counts.reshape(-1)
    rank = np.empty(E, np.int64)
    rank[order] = np.arange(E) - starts[key[order]]
    base_arr = np.zeros((NCORES, TILES, 2), np.int64)
    for ti in range(TILES):
        for g in range(2):
            base_arr[:, ti, g] = slot_base[(ti, g)]
    slot_of_edge = base_arr.reshape(-1)[key] + rank

    idx_val = np.where(grp_b == 0, src_p, src_p - HALF_A)

    gidx = np.zeros((NCORES, 128, SLOTS // 16), np.int16)
    dstrel = np.full((NCORES, 128, SLOTS // 128), -1.0, np.float32)
    NGRP = (SLOTS // 128) // 4
    easl = np.zeros((NCORES, SLOTS, EF), np.float32)
    for c in range(NCORES):
        m = core == c
        sl = slot_of_edge[m]
        dstrel[c][sl % 128, sl // 128] = drel[m]
        easl[c][sl] = edge_attr[m]
        iv = np.zeros(SLOTS, np.int64)
        iv[sl] = idx_val[m]
        col0 = 0
        for tl in chunk_tiles:
            for g, BG in ((0, BA), (1, BB)):
                L = len(tl) * BG * 128
                s_lo = slot_base[(tl[0], g)]
                j = np.arange(L)
                gidx[c][j % 16, col0 + j // 16] = iv[s_lo:s_lo + L].astype(np.int16)
                col0 += L // 16
        gidx[c] = np.tile(gidx[c][:16], (8, 1))

    dstrelT = np.zeros((NCORES, 8, NGRP, 128), np.float32)
    for c in range(NCORES):
        dq = dstrel[c].T.reshape(NGRP, 4, 128)   # [g, q, p]
        dstrelT[c, 0:4] = dq.transpose(1, 0, 2)
        dstrelT[c, 4] = 1.0
    tgen_rhs = np.zeros((8, 512), np.float32)
    for q in range(4):
        tgen_rhs[q, 128 * q:128 * (q + 1)] = 1.0
    tgen_rhs[4] = -np.tile(np.arange(128, dtype=np.float32), 4)

    # eaT2 stacked pairing: unit u covers slots [256u,256u+128) top, +128 bottom
    easl_u = easl.reshape(NCORES, SLOTS // 256, 2, 128, EF)
    eaT2 = np.zeros((NCORES, 2 * EF, SLOTS // 2), BF16)
    for c in range(NCORES):
        eaT2[c, :EF] = easl_u[c, :, 0].transpose(2, 0, 1).reshape(EF, -1).astype(BF16)
        eaT2[c, EF:] = easl_u[c, :, 1].transpose(2, 0, 1).reshape(EF, -1).astype(BF16)

    # node-side tensors
    xpad = np.zeros((N, DIM), np.float32)
    xpad[:, :XF] = x
    T1 = np.zeros((NPAD, DIM), np.float32)
    flat = slot2node.reshape(-1)
    valid = flat >= 0
    cores_flat = np.repeat(np.arange(NCORES), NLOC)
    slots_flat = np.tile(np.arange(NLOC), NCORES)
    T1[_pad_id(cores_flat[valid], slots_flat[valid])] = xpad[flat[valid]]
    hT0 = np.zeros((NCORES, DIM, NLOC), BF16)
    # pooling: poh[c, n_rel, ti, j] = 1/cnt[g] where j = g - g0_c is the
    # core-local graph index (each core's nodes span < 128 graphs), and
    # pw[c, j, g] = 1 places local column j at global graph g0_c + j.
    # Mean pooling is baked into poh; the AllReduce adds cross-core parts.
    cnt = np.bincount(batch, minlength=G).astype(np.float32)
    inv_cnt = 1.0 / np.maximum(cnt, 1.0)
    poh = np.zeros((NCORES, 128, TILES, 128), np.float32)
    pw = np.zeros((NCORES, 128, G), BF16)
    for c in range(NCORES):
        sn = slot2node[c]
        v = sn >= 0
        hn = np.zeros((NLOC, DIM), np.float32)
        hn[v] = xpad[sn[v]]
        hT0[c] = hn.T.astype(BF16)
        vv = v.nonzero()[0]
        gs = batch[sn[vv]]
        g0 = int(gs.min())
        assert int(gs.max()) - g0 < 128, "core graph span exceeds 128"
        poh[c][vv % 128, vv // 128, gs - g0] = inv_cnt[gs]
        nj = min(128, G - g0)
        pw[c][np.arange(nj), g0 + np.arange(nj)] = 1.0
    poh = poh.astype(BF16)

    def padw(a, r, cc):
        out = np.zeros((r, cc), np.float32)
        a = np.asarray(a, np.float32)
        out[:a.shape[0], :a.shape[1]] = a
        return out

    wb = {}
    for li, p in ((1, "c1"), (2, "c2"), (3, "c3")):
        be1 = padw(inputs[f"{p}_be1"], EF, DIM)
        be2 = padw(inputs[f"{p}_be2"], DIM, DIM)
        be1_2 = np.zeros((2 * EF, 128), np.float32)
        be1_2[:EF, :DIM] = be1
        be1_2[EF:, DIM:] = be1
        be2_2 = np.zeros((128, 128), np.float32)
        be2_2[:DIM, :DIM] = be2
        be2_2[DIM:, DIM:] = be2
        wb[f"be1_{li}"] = be1_2.astype(BF16)
        wb[f"be2_{li}"] = be2_2.astype(BF16)
        wb[f"m1_{li}"] = padw(inputs[f"{p}_m1"], DIM, DIM).astype(BF16)
        wb[f"m2_{li}"] = padw(inputs[f"{p}_m2"], DIM, DIM).astype(BF16)
    eps = [float(np.asarray(inputs[f"{p}_eps"]).reshape(-1)[0])
           for p in ("c1", "c2", "c3")]
    epsv = np.array([[eps[0]], [eps[1]], [eps[2]], [eps[2]]], np.float32)

    # fc1 weights laid out to match the stacked pooled tensor:
    # pooled group k holds layers (2k, 2k+1) at partitions (0:64, 64:128).
    fc1_in = np.asarray(inputs["fc1_w"], np.float32)      # [256, 64]
    fc1_r = np.zeros((128, 2, DIM), np.float32)
    for l in range(NLAYERS):
        fc1_r[64 * (l % 2):64 * (l % 2) + 64, l // 2, :] = \
            fc1_in[64 * l:64 * (l + 1), :]

    common = {
        "t0": T1,
        "idbf": np.eye(128, dtype=np.float32).astype(BF16),
        "epsv": epsv,
        "tgen_rhs": tgen_rhs.astype(BF16),
        "fc1_w": fc1_r,
        "fc1_b": np.asarray(inputs["fc1_b"], np.float32).reshape(DIM, 1),
        "fc2_w": np.asarray(inputs["fc2_w"], np.float32),
        "fc2_b": np.asarray(inputs["fc2_b"], np.float32).reshape(DIM, 1),
        "fc3_w": np.asarray(inputs["fc3_w"], np.float32),
        "fc3_b": np.asarray(inputs["fc3_b"], np.float32).reshape(DIM, 1),
        "fc4_w": np.asarray(inputs["fc4_w"], np.float32),
        "fc4_b": np.asarray(inputs["fc4_b"], np.float32).reshape(1, 1),
    }
    common.update(wb)
    for i in range(1, 5):
        for s in "gbmv":
            common[f"bn{i}_{s}"] = np.asarray(inputs[f"bn{i}_{s}"],
                                              np.float32).reshape(DIM, 1)

    in_maps = []
    for c in range(NCORES):
        m = dict(common)
        m["gidx"] = gidx[c]
        m["dstrelT"] = dstrelT[c].astype(BF16)
        m["eaT2"] = eaT2[c]
        m["hT0"] = hT0[c]
        m["poh"] = poh[c]
        m["pw"] = pw[c]
        in_maps.append(m)

    struct = dict(BA=BA, BB=BB, NBT=NBT, NBLK=NBLK, SLOTS=SLOTS,
                  chunk_tiles=chunk_tiles, slot_base=slot_base)
    return in_maps, struct


# ------------------------------------------------------------- bass program --

def _build(struct):
    from concourse import bacc, tile, mybir
    f32, bf16, i16 = mybir.dt.float32, mybir.dt.bfloat16, mybir.dt.int16
    Alu = mybir.AluOpType
    Act = mybir.ActivationFunctionType

    BA, BB, NBT = struct["BA"], struct["BB"], struct["NBT"]
    SLOTS = struct["SLOTS"]
    chunk_tiles = struct["chunk_tiles"]
    NB = TPC * NBT            # max blocks per chunk

    nc = bacc.Bacc("TRN2", target_bir_lowering=False, debug=False,
                   num_devices=NCORES, num_swdge_queues=4)

    def din(name, shape, dt=f32):
        return nc.dram_tensor(name, shape, dt, kind="ExternalInput")

    t0 = din("t0", [NPAD, DIM])
    gidx_d = din("gidx", [128, SLOTS // 16], i16)
    NGRP = (SLOTS // 128) // 4
    dstrelT_d = din("dstrelT", [8, NGRP, 128], bf16)
    tgen_d = din("tgen_rhs", [8, 512], bf16)
    eaT2_d = din("eaT2", [2 * EF, SLOTS // 2], bf16)
    hT0_d = din("hT0", [DIM, NLOC], bf16)
    poh_d = din("poh", [128, TILES, 128], bf16)
    pw_d = din("pw", [128, G], bf16)
    idbf_d = din("idbf", [128, 128], bf16)
    epsv_d = din("epsv", [4, 1])
    wdict = {}
    for li in (1, 2, 3):
        wdict[f"be1_{li}"] = din(f"be1_{li}", [2 * EF, 128], bf16)
        wdict[f"be2_{li}"] = din(f"be2_{li}", [128, 128], bf16)
        wdict[f"m1_{li}"] = din(f"m1_{li}", [DIM, DIM], bf16)
        wdict[f"m2_{li}"] = din(f"m2_{li}", [DIM, DIM], bf16)
    for i in range(1, 5):
        for s in "gbmv":
            wdict[f"bn{i}_{s}"] = din(f"bn{i}_{s}", [DIM, 1])
    fc1_w = din("fc1_w", [128, 2, DIM])
    fc2_w = din("fc2_w", [DIM, DIM])
    fc3_w = din("fc3_w", [DIM, DIM])
    fc4_w = din("fc4_w", [DIM, 1])
    fcb_d = {"b1": din("fc1_b", [DIM, 1]), "b2": din("fc2_b", [DIM, 1]),
             "b3": din("fc3_b", [DIM, 1]), "b4": din("fc4_b", [1, 1])}

    out_d = nc.dram_tensor("out", [1, G], f32, kind="ExternalOutput")
    bounce = [nc.dram_tensor(f"bounce{l}", [NLOC, DIM], f32)
              for l in range(NLAYERS - 1)]
    tables = [t0] + [nc.dram_tensor(f"T{l}", [NPAD, DIM], f32,
                                    addr_space="Shared")
                     for l in (1, 2, 3)]
    arin_d = [nc.dram_tensor(f"arin{k}", [128, G], f32) for k in (0, 1)]
    arout_d = [nc.dram_tensor(f"arout{k}", [128, G], f32, addr_space="Shared")
               for k in (0, 1)]

    qctr = [0]

    def next_q():
        q = qctr[0] % 4
        qctr[0] += 1
        return q

    with tile.TileContext(nc) as tc:
        with tc.tile_pool(name="res", bufs=1) as res, \
             tc.tile_pool(name="hsrcp", bufs=2) as hsrcp, \
             tc.tile_pool(name="msgp", bufs=2) as msgp, \
             tc.tile_pool(name="e1p", bufs=2) as e1p, \
             tc.tile_pool(name="eap", bufs=3) as eap, \
             tc.tile_pool(name="smallp", bufs=2) as smallp, \
             tc.tile_pool(name="psA", bufs=2, space="PSUM") as psA, \
             tc.tile_pool(name="psB", bufs=1, space="PSUM") as psB, \
             tc.tile_pool(name="psP", bufs=2, space="PSUM") as psP, \
             tc.tile_pool(name="psG", bufs=2, space="PSUM") as psG, \
             tc.tile_pool(name="psL", bufs=1, space="PSUM") as psL:

            # ---------------- residents
            def load(name, shape, dt, dram):
                tl_ = res.tile(shape, dt, tag=name)
                nc.sync.dma_start(out=tl_[:], in_=dram[:])
                return tl_

            tgen_sb = load("tgen", [8, 512], bf16, tgen_d)
            idbf_sb = load("idbf", [128, 128], bf16, idbf_d)
            poh_sb = res.tile([128, TILES, 128], bf16, tag="poh")
            nc.scalar.dma_start(out=poh_sb[:], in_=poh_d[:])
            pw_sb = res.tile([128, G], bf16, tag="pw")
            nc.scalar.dma_start(out=pw_sb[:], in_=pw_d[:])
            def loads(name, shape, dt, dram):
                tl_ = res.tile(shape, dt, tag=name)
                nc.scalar.dma_start(out=tl_[:], in_=dram[:])
                return tl_

            w_sb = {k: loads(f"w_{k}", list(d.shape), d.dtype, d)
                    for k, d in wdict.items()}
            fc1w_sb = loads("fc1w", [128, 2, DIM], f32, fc1_w)
            fc2w_sb = loads("fc2w", [DIM, DIM], f32, fc2_w)
            fc3w_sb = loads("fc3w", [DIM, DIM], f32, fc3_w)
            fc4w_sb = loads("fc4w", [DIM, 1], f32, fc4_w)
            fcb_sb = {k: loads(f"fcb{k}", list(d.shape), f32, d)
                      for k, d in fcb_d.items()}
            hN = res.tile([128, TILES, DIM], bf16, tag="hN")
            aggT = res.tile([DIM, NLOC], bf16, tag="aggT")
            hT = res.tile([DIM, NLOC], bf16, tag="hT")
            nc.scalar.dma_start(out=hT[:], in_=hT0_d[:])
            arin_sb = res.tile([128, 2, G], f32, tag="arin")

            # eps broadcast [128,1] per layer: (1+eps)
            eps1p = []
            for l in range(NLAYERS):
                e0 = res.tile([1, 1], f32, tag=f"eps0_{l}")
                nc.sync.dma_start(out=e0[:], in_=epsv_d[l:l + 1, :])
                eb = res.tile([128, 1], f32, tag=f"epsb{l}")
                nc.gpsimd.partition_broadcast(eb[:], e0[:], channels=128)
                e1 = res.tile([128, 1], f32, tag=f"eps1p{l}")
                nc.vector.tensor_scalar_add(e1[:], eb[:], 1.0)
                eps1p.append(e1)

            # bn params -> scale g', shift b'
            bn_s, bn_t = [], []
            for i in range(1, 5):
                v = w_sb[f"bn{i}_v"]; gg = w_sb[f"bn{i}_g"]
                bb = w_sb[f"bn{i}_b"]; mm = w_sb[f"bn{i}_m"]
                ve = res.tile([DIM, 1], f32, tag=f"bnve{i}")
                nc.vector.tensor_scalar_add(ve[:], v[:], 1e-5)
                sq = res.tile([DIM, 1], f32, tag=f"bnsq{i}")
                nc.scalar.activation(sq[:], ve[:], Act.Sqrt)
                inv = res.tile([DIM, 1], f32, tag=f"bninv{i}")
                nc.vector.reciprocal(inv[:], sq[:])
                gp = res.tile([DIM, 1], f32, tag=f"bngp{i}")
                nc.vector.tensor_mul(gp[:], gg[:], inv[:])
                tt = res.tile([DIM, 1], f32, tag=f"bntt{i}")
                nc.vector.tensor_mul(tt[:], mm[:], gp[:])
                bp = res.tile([DIM, 1], f32, tag=f"bnbp{i}")
                nc.vector.tensor_sub(bp[:], bb[:], tt[:])
                bn_s.append(gp); bn_t.append(bp)

            # ---------------- layers (software-pipelined edge + node chunks)
            for l in range(NLAYERS):
                wl = min(l + 1, 3)
                be1 = w_sb[f"be1_{wl}"]; be2 = w_sb[f"be2_{wl}"]
                m1 = w_sb[f"m1_{wl}"]; m2 = w_sb[f"m2_{wl}"]
                tbl = tables[l]
                bl = bounce[l] if l < NLAYERS - 1 else None
                pool_ps = psL.tile([128, DIM], f32, tag="plg")

                def node_work(tl):
                    # node update for a chunk whose scatter already finished
                    ntl = len(tl)
                    gw = ntl * 128
                    sl = slice(128 * tl[0], 128 * tl[0] + gw)
                    zt = smallp.tile([DIM, 512], bf16, tag="zt")
                    nc.vector.scalar_tensor_tensor(
                        zt[:, 0:gw], hT[:, sl], eps1p[l][0:DIM, :],
                        aggT[:, sl], Alu.mult, Alu.add)
                    ps1 = psA.tile([128, 512], f32, tag="ps1")
                    nc.tensor.matmul(ps1[0:DIM, 0:gw], m1[:], zt[:, 0:gw],
                                     start=True, stop=True)
                    r1 = smallp.tile([DIM, 512], bf16, tag="r1")
                    nc.scalar.activation(r1[:, 0:gw], ps1[0:DIM, 0:gw], Act.Relu)
                    ps2 = psB.tile([128, 512], f32, tag="ps2")
                    nc.tensor.matmul(ps2[0:DIM, 0:gw], m2[:], r1[:, 0:gw],
                                     start=True, stop=True)
                    rr = smallp.tile([DIM, 512], bf16, tag="rr")
                    nc.scalar.activation(rr[:, 0:gw], ps2[0:DIM, 0:gw], Act.Relu)
                    nc.vector.tensor_scalar(hT[:, sl], rr[:, 0:gw], bn_s[l][:],
                                            bn_t[l][:], Alu.mult, Alu.add)
                    for ti in tl:
                        pth = psP.tile([128, DIM], bf16, tag="pse", name="pth")
                        nc.tensor.transpose(pth[:],
                                            hT[:, 128 * ti:128 * (ti + 1)],
                                            idbf_sb[0:DIM, 0:DIM])
                        nc.vector.tensor_copy(hN[:, ti, :], pth[:])
                        if bl is not None:
                            hn32 = smallp.tile([128, DIM], f32, tag="hn32")
                            nc.vector.tensor_copy(hn32[:], pth[:])
                            nc.scalar.dma_start(
                                out=bl[128 * ti:128 * (ti + 1), :], in_=hn32[:])
                        nc.tensor.matmul(pool_ps[:], poh_sb[:, ti, :],
                                         hN[:, ti, :],
                                         start=(ti == 0), stop=(ti == TILES - 1))

                for ci, tl in enumerate(chunk_tiles):
                    ntl = len(tl)
                    nb = ntl * NBT
                    ncols = nb * 64
                    s0 = struct["slot_base"][(tl[0], 0)]
                    assert s0 % 256 == 0
                    la = ntl * BA * 128
                    lb = ntl * BB * 128
                    # gather idx staging + gathers (A then B table half)
                    gix = smallp.tile([128, TPC * NBT * 8], i16, tag="gix",
                                      bufs=6)
                    nc.sync.dma_start(out=gix[:, 0:nb * 8],
                                      in_=gidx_d[:, s0 // 16:(s0 + la + lb) // 16])
                    hsrc = hsrcp.tile([128, NB, DIM], f32, tag="hsrc")
                    nc.gpsimd.dma_gather(
                        out_ap=hsrc[:, 0:ntl * BA, :], in_ap=tbl[0:HALF_A, :],
                        idxs_ap=gix[:, 0:la // 16],
                        num_idxs=la, num_idxs_reg=la, elem_size=DIM,
                        single_packet=False, queue_num=next_q())
                    nc.gpsimd.dma_gather(
                        out_ap=hsrc[:, ntl * BA:nb, :], in_ap=tbl[HALF_A:NPAD, :],
                        idxs_ap=gix[:, la // 16:(la + lb) // 16],
                        num_idxs=lb, num_idxs_reg=lb, elem_size=DIM,
                        single_packet=False, queue_num=next_q())

                    # bond encoder stage 1 for this chunk
                    c0 = s0 // 2
                    ea_sb = eap.tile([2 * EF, TPC * NBT * 64], bf16, tag="ea")
                    nc.sync.dma_start(out=ea_sb[:, 0:ncols],
                                      in_=eaT2_d[:, c0:c0 + ncols])
                    e1t = e1p.tile([128, TPC * NBT * 64], bf16, tag="e1")
                    g0 = 0
                    while g0 < ncols:
                        gw = min(512, ncols - g0)
                        ps1 = psA.tile([128, 512], f32, tag="ps1")
                        nc.tensor.matmul(ps1[:, 0:gw], be1[:], ea_sb[:, g0:g0 + gw],
                                         start=True, stop=True)
                        nc.scalar.activation(e1t[:, g0:g0 + gw], ps1[:, 0:gw],
                                             Act.Relu)
                        g0 += gw

                    # stage 2 (pair matmul -> e natural in PSUM banks of 8 blks)
                    # + msg = hsrc + e, relu on ACT in place
                    assert nb % 2 == 0
                    msg = msgp.tile([128, NB, DIM], bf16, tag="msg")
                    for v4 in range((nb + 7) // 8):
                        bcnt = min(8, nb - 8 * v4)
                        pse = psP.tile([128, 8, DIM], f32, tag="pse")
                        for j in range(bcnt // 2):
                            u = 4 * v4 + j
                            nc.tensor.matmul(pse[:, 2 * j:2 * j + 2, :],
                                             e1t[:, 128 * u:128 * (u + 1)],
                                             be2[:], start=True, stop=True)
                        nc.vector.tensor_add(msg[:, 8 * v4:8 * v4 + bcnt, :],
                                             hsrc[:, 8 * v4:8 * v4 + bcnt, :],
                                             pse[:, 0:bcnt, :])
                    nc.scalar.activation(msg[:, 0:nb, :], msg[:, 0:nb, :], Act.Relu)

                    # one-hot generation: t = dstrel - n via PE, is_eq(imm 0)
                    assert nb % 4 == 0
                    dsl = smallp.tile([8, TPC * NBT // 4, 128], bf16, tag="dsl",
                                      bufs=3)
                    g4 = s0 // 512
                    ng = nb // 4
                    nc.scalar.dma_start(out=dsl[:, 0:ng, :],
                                        in_=dstrelT_d[:, g4:g4 + ng, :])
                    ohs = []
                    for gq in range(ng):
                        pst4 = psA.tile([128, 512], f32, tag="ps1", name="pst4")
                        nc.tensor.matmul(pst4[:], dsl[:, gq, :], tgen_sb[:],
                                         start=True, stop=True)
                        oh = smallp.tile([128, 512], bf16, tag="oh", bufs=8)
                        nc.vector.tensor_single_scalar(oh[:], pst4[:], 0.0,
                                                       Alu.is_equal)
                        ohs.append(oh)

                    # scatter: msg stationary, one-hot streamed 128-wide;
                    # accumulates feature-major [feat, dst] agg directly.
                    for k, ti in enumerate(tl):
                        aps = psG.tile([DIM, 128], f32, tag="aggps")
                        blocks = ([k * BA + i for i in range(BA)] +
                                  [ntl * BA + k * BB + i for i in range(BB)])
                        for j, b in enumerate(blocks):
                            nc.tensor.matmul(aps[:], msg[:, b, :],
                                             ohs[b // 4][:, 128 * (b % 4):
                                                         128 * (b % 4) + 128],
                                             start=(j == 0), stop=(j == NBT - 1))
                        nc.vector.tensor_copy(
                            aggT[:, 128 * ti:128 * (ti + 1)], aps[:])

                # node phase: per-chunk node updates, then one AllGather
                for ci, tl in enumerate(chunk_tiles):
                    node_work(tl)
                if bl is not None:
                    nc.gpsimd.collective_compute(
                        "AllGather", Alu.bypass,
                        replica_groups=[list(range(NCORES))],
                        ins=[bl[:]], outs=[tables[l + 1][:]])
                # drain pooled locals -> placement -> arin
                pgl = smallp.tile([128, DIM], bf16, tag="pgl")
                nc.vector.tensor_copy(pgl[:], pool_ps[:])
                plc_ps = psB.tile([DIM, G], f32, tag="ps2", name="plp")
                nc.tensor.matmul(plc_ps[:], pgl[:], pw_sb[:],
                                 start=True, stop=True)
                nc.vector.tensor_copy(
                    arin_sb[64 * (l % 2):64 * (l % 2) + 64, l // 2, :],
                    plc_ps[:])
                if l == 1:
                    # layers 0-1 pooled: reduce now, hidden under layer 2
                    nc.sync.dma_start(out=arin_d[0][:], in_=arin_sb[:, 0, :])
                    nc.gpsimd.collective_compute(
                        "AllReduce", Alu.add,
                        replica_groups=[list(range(NCORES))],
                        ins=[arin_d[0][:]], outs=[arout_d[0][:]])

            # ---------------- epilogue: second AllReduce half + head MLP
            nc.sync.dma_start(out=arin_d[1][:], in_=arin_sb[:, 1, :])
            nc.gpsimd.collective_compute(
                "AllReduce", Alu.add, replica_groups=[list(range(NCORES))],
                ins=[arin_d[1][:]], outs=[arout_d[1][:]])
            pf = res.tile([128, 2, G], f32, tag="pf")
            nc.sync.dma_start(out=pf[:, 0, :], in_=arout_d[0][:])
            nc.sync.dma_start(out=pf[:, 1, :], in_=arout_d[1][:])

            hps = psA.tile([128, 512], f32, tag="ps1")
            for k in range(2):
                nc.tensor.matmul(hps[0:DIM, :], fc1w_sb[:, k, :], pf[:, k, :],
                                 start=(k == 0), stop=(k == 1))
            h1 = res.tile([DIM, 512], f32, tag="h1")
            nc.scalar.activation(h1[:], hps[0:DIM, :], Act.Relu,
                                 bias=fcb_sb["b1"][:])
            hps2 = psB.tile([128, 512], f32, tag="ps2")
            nc.tensor.matmul(hps2[0:DIM, :], fc2w_sb[:], h1[:], start=True, stop=True)
            h2 = res.tile([DIM, 512], f32, tag="h2")
            nc.scalar.activation(h2[:], hps2[0:DIM, :], Act.Relu,
                                 bias=fcb_sb["b2"][:])
            hps3 = psA.tile([128, 512], f32, tag="ps1")
            nc.tensor.matmul(hps3[0:DIM, :], fc3w_sb[:], h2[:], start=True, stop=True)
            h3 = res.tile([DIM, 512], f32, tag="h3")
            nc.scalar.activation(h3[:], hps3[0:DIM, :], Act.Relu,
                                 bias=fcb_sb["b3"][:])
            hps4 = psB.tile([128, 512], f32, tag="ps2")
            nc.tensor.matmul(hps4[0:1, :], fc4w_sb[:], h3[:], start=True, stop=True)
            ho = res.tile([1, G], f32, tag="ho")
            nc.scalar.activation(ho[:], hps4[0:1, :], Act.Identity,
                                 bias=fcb_sb["b4"][:])
            nc.sync.dma_start(out=out_d[:], in_=ho[:])

    nc.compile()
    return nc


# ------------------------------------------------------------------ runner --

_CACHE = {}


def kernel(**inputs):
    from concourse.bass_utils import run_bass_kernel_spmd
    in_maps, struct = _prep(inputs)
    key = (struct["BA"], struct["BB"])
    if key not in _CACHE:
        _CACHE[key] = _build(struct)
    nc = _CACHE[key]
    trace = os.environ.get("BASSGIN_TRACE", "0") == "1"
    res = run_bass_kernel_spmd(nc, in_maps, core_ids=list(range(NCORES)),
                               trace=trace)
    kernel.last_result = res
    out = res.results[0]["out"].reshape(G).astype(np.float32)
    return out
